# revision 72
# baseline (speedup 1.0000x reference)
"""Trainium2 Bass kernel for per-sample Brownian-distance-covariance (BDC) pooling.

Problem: x [128, 640, 100] f32, t [1,1] f32 (log temperature).
  per sample: G = x @ x^T; dcov = d_i + d_j - 2G; dcov = max(dcov, 1e-4);
  z = sqrt(exp(t)*dcov + 1e-5); out = z - rowmean - colmean + totmean.
Output: [128, 409600] f32.

Strategy (8 NeuronCores, pure data parallel, 16 samples/core):
  - Row-coalesced input layout: partition p holds dims 5p..5p+4, so the
    f32->bf16 cast load is one big-descriptor DMA per group of 2 samples.
  - d = ||x_i||^2 via Pool-engine square + DVE segmented reduce (keeps the
    Activation engine free for the sqrt pass, which is near its roofline).
  - Gram via TensorE; the d_j row enters the same PSUM accumulation through
    constant one-hot "selector" matmuls (SEL_j @ t5 broadcasts hi+lo row j
    of the transposed bf16 hi/lo split across all partitions) - no
    SBUF->SBUF pack DMAs anywhere.  d_i enters via the per-partition
    activation bias, which also compensates bf16 rounding exactly on the
    diagonal, so no clamp is needed.
  - The colmean broadcast for double centering is built the same way from
    the transposed rowmean hi/lo split (selector matmuls into PSUM).
  - Fully per-sample software pipeline: squares run 4 samples ahead, heads
    (d/hi-lo/hrow/xT) 2 ahead, and the tail of sample n-2 (double
    centering + store) overlaps gram+sqrt of sample n, so every
    cross-engine chain has a full iteration of slack.  Per-chunk output
    DMAs keep the serialized DMA engine interleaving small critical
    transfers.  xT PSUM->SBUF copies ride on the Activation engine to
    offload DVE, which is the pacing engine.
  - Double centering: one scalar_tensor_tensor per chunk on DVE, reading
    the colmean broadcast straight from PSUM; the output AP de-permutes
    columns so each sample's result is DMA'd from contiguous SBUF.
"""
import numpy as np
from contextlib import ExitStack

import concourse.bass as bass
import concourse.bacc as bacc
import concourse.tile as tile
from concourse import mybir
from concourse.bass_utils import run_bass_kernel_spmd

F32 = mybir.dt.float32
BF16 = mybir.dt.bfloat16
AF = mybir.ActivationFunctionType
OP = mybir.AluOpType

N_CORES = 8
B_TOTAL = 128
B_CORE = B_TOTAL // N_CORES  # 16
DIM = 640
M = 100
NCHUNK = DIM // 128  # 5
GSZ = 2
NG = B_CORE // GSZ  # 8 groups

_cached_nc = None


def build():
    nc = bacc.Bacc("TRN2", target_bir_lowering=False)
    x = nc.dram_tensor("x", [B_CORE, DIM, M], F32, kind="ExternalInput")
    consts = nc.dram_tensor("consts", [128, 2], F32, kind="ExternalInput")
    ident_in = nc.dram_tensor("ident", [128, 128], F32, kind="ExternalInput")
    sel_in = nc.dram_tensor("sel", [2 * NCHUNK, NCHUNK * 128], F32, kind="ExternalInput")
    out = nc.dram_tensor("out", [B_CORE, DIM * DIM], F32, kind="ExternalOutput")

    with tile.TileContext(nc) as tc, ExitStack() as ctx:
        const_p = ctx.enter_context(tc.tile_pool(name="const", bufs=1))
        xbp = ctx.enter_context(tc.tile_pool(name="xbp", bufs=4))
        sqp = ctx.enter_context(tc.tile_pool(name="sqp", bufs=8))
        hp = ctx.enter_context(tc.tile_pool(name="hp", bufs=8))
        xtp = ctx.enter_context(tc.tile_pool(name="xtp", bufs=5))
        zp = ctx.enter_context(tc.tile_pool(name="zp", bufs=7))
        opool = ctx.enter_context(tc.tile_pool(name="op", bufs=3))
        pk = ctx.enter_context(tc.tile_pool(name="pk", bufs=6))
        psamp = ctx.enter_context(tc.tile_pool(name="psamp", bufs=6))
        ps_g = ctx.enter_context(tc.tile_pool(name="psg", bufs=2, space="PSUM"))
        ps_m = ctx.enter_context(tc.tile_pool(name="psm", bufs=1, space="PSUM"))
        ps_x = ctx.enter_context(tc.tile_pool(name="psx", bufs=2, space="PSUM"))

        # ---- input prefetch first so sample 0's chain starts ASAP ----
        def in_dma(g):
            b0 = GSZ * g
            xbg = xbp.tile([128, GSZ, NCHUNK, M], BF16, tag="xb")
            nc.gpsimd.dma_start(
                xbg[:],
                x[b0 : b0 + GSZ].rearrange("s (p r) m -> p s r m", p=128),
            )
            return xbg

        with tc.high_priority():
            xbg0 = in_dma(0)

        # ---- constants ----
        c_consts = const_p.tile([128, 2], F32)
        nc.sync.dma_start(c_consts[:], consts[:])
        neg2alpha = c_consts[:, 0:1]
        twoalpha = c_consts[:, 1:2]

        c_ident = const_p.tile([128, 128], BF16)
        nc.gpsimd.dma_start(c_ident[:], ident_in[:])

        c_ones128 = const_p.tile([128, 128], F32)
        nc.vector.memset(c_ones128[:], 1.0)
        atl_warm = const_p.tile([1, 1], F32)
        nc.scalar.activation(atl_warm[:], c_ones128[0:1, 0:1], AF.Sqrt)
        # selector weights: SEL_j = c_sel[:, j*128:(j+1)*128] is [2*NCHUNK,128]
        # with ones in rows j and NCHUNK+j -> matmul broadcasts (hi+lo) row j
        # of a [2*NCHUNK,128] tile across all 128 output partitions.
        c_sel = const_p.tile([2 * NCHUNK, NCHUNK * 128], BF16)
        nc.gpsimd.dma_start(c_sel[:], sel_in[:])

        def emit_sq(n, xbg, eng=None):
            bp = n % GSZ
            sqs = sqp.tile([128, NCHUNK, M], F32, tag="sq")
            (eng or nc.gpsimd).tensor_mul(sqs[:], xbg[:, bp], xbg[:, bp])
            return sqs

        def emit_head(n, xbg, sqs):
            """Per-sample head: d, hi/lo split, hrow pack, xT."""
            bp = n % GSZ
            # transposes -> xT [100, 640]
            xps = ps_x.tile([M, DIM], BF16, tag="xps")
            for r in range(NCHUNK):
                nc.tensor.transpose(
                    xps[:, r * 128 : (r + 1) * 128], xbg[:, bp, r, :], c_ident[:]
                )
            xT = xtp.tile([M, DIM], BF16, tag="xT")
            nc.scalar.copy(xT[:], xps[:])
            d_s = hp.tile([128, NCHUNK], F32, tag="d")
            nc.vector.tensor_reduce(
                d_s[:], sqs[:], axis=mybir.AxisListType.X, op=OP.add
            )
            # hi/lo split of -0.5*d
            hstack = hp.tile([128, 2 * NCHUNK], BF16, tag="hstack")
            nc.vector.tensor_scalar(
                out=hstack[:, 0:NCHUNK], in0=d_s[:], scalar1=-0.5, scalar2=None,
                op0=OP.mult,
            )
            hres = hp.tile([128, NCHUNK], F32, tag="hres")
            nc.vector.tensor_scalar(
                out=hres[:], in0=d_s[:], scalar1=-0.5, scalar2=None, op0=OP.mult
            )
            nc.vector.tensor_sub(
                hstack[:, NCHUNK : 2 * NCHUNK], hres[:], hstack[:, 0:NCHUNK]
            )
            # transpose hi/lo stack, pack [2, 640] row
            xps2 = ps_x.tile([M, DIM], BF16, tag="xps")
            nc.tensor.transpose(
                xps2[0 : 2 * NCHUNK, 0:128], hstack[:], c_ident[:]
            )
            t5 = hp.tile([2 * NCHUNK, 128], BF16, tag="t5")
            nc.vector.tensor_copy(t5[:], xps2[0 : 2 * NCHUNK, 0:128])
            tmpb = hp.tile([128, NCHUNK], F32, tag="tmpb")
            nc.vector.tensor_add(tmpb[:], d_s[:], hstack[:, 0:NCHUNK])
            nc.vector.tensor_add(tmpb[:], tmpb[:], hstack[:, NCHUNK : 2 * NCHUNK])
            bias_s = hp.tile([128, NCHUNK], F32, tag="bias")
            nc.vector.tensor_scalar(
                out=bias_s[:], in0=tmpb[:], scalar1=twoalpha, scalar2=1e-5,
                op0=OP.mult, op1=OP.add,
            )
            rowsum_s = hp.tile([128, NCHUNK], F32, tag="rowsum")
            return xT, bias_s, t5, rowsum_s

        def tail_prep_a(st):
            """Early tail part: rowmean chain + t10 transpose/copy."""
            rowsum_s = st["rowsum"]
            rm_s = psamp.tile([128, NCHUNK], F32, tag="rm")
            rs_acc = psamp.tile([128, 1], F32, tag="rs")
            nc.vector.tensor_scalar(
                out=rm_s[:], in0=rowsum_s[:], scalar1=1.0 / DIM, scalar2=0.0,
                op0=OP.mult, op1=OP.add, accum_out=rs_acc[:],
            )
            rmstack = psamp.tile([128, 2 * NCHUNK], BF16, tag="rmstack")
            nc.vector.tensor_copy(rmstack[:, 0:NCHUNK], rm_s[:])
            nc.vector.tensor_sub(
                rmstack[:, NCHUNK : 2 * NCHUNK], rm_s[:], rmstack[:, 0:NCHUNK]
            )
            xps3 = ps_x.tile([M, DIM], BF16, tag="xps")
            nc.tensor.transpose(
                xps3[0 : 2 * NCHUNK, 0:128], rmstack[:], c_ident[:]
            )
            t10 = psamp.tile([2 * NCHUNK, 128], BF16, tag="t10")
            nc.vector.tensor_copy(t10[:], xps3[0 : 2 * NCHUNK, 0:128])
            st["t10"], st["rm"], st["rsacc"] = t10, rm_s, rs_acc

        def tail_prep_b(st):
            t10, rm_s, rs_acc = st["t10"], st["rm"], st["rsacc"]
            mps = ps_m.tile([128, DIM], F32, tag="mps")
            nc.tensor.matmul(
                mps[:, 0:1], c_ones128[:], rs_acc[:],
                start=True, stop=True, skip_group_check=True,
            )
            tm_b = psamp.tile([128, 1], F32, tag="tm")
            nc.vector.tensor_scalar(
                out=tm_b[:], in0=mps[:, 0:1], scalar1=1.0 / DIM,
                scalar2=None, op0=OP.mult,
            )
            s0_b = psamp.tile([128, NCHUNK], F32, tag="s0")
            nc.vector.tensor_scalar(
                out=s0_b[:], in0=rm_s[:], scalar1=tm_b[:], scalar2=None,
                op0=OP.subtract,
            )
            for j in range(NCHUNK):
                nc.tensor.matmul(
                    mps[:, j * 128 : (j + 1) * 128],
                    c_sel[:, j * 128 : (j + 1) * 128], t10[:],
                    start=True, stop=True, skip_group_check=True,
                )
            st["mps"], st["s0"] = mps, s0_b

        def tail_stt(st):
            b, z = st["n"], st["z"]
            mps, s0_b = st["mps"], st["s0"]
            outt = opool.tile([128, NCHUNK, DIM], F32, tag="outt")
            mv = mps[:].rearrange("p (a b) -> p a b", a=NCHUNK)
            for r in range(NCHUNK):
                zv = z[:, r, :].rearrange("p (a b) -> p a b", a=NCHUNK)
                ov = outt[:, r, :].rearrange("p (b f) -> p f b", f=NCHUNK)
                nc.vector.scalar_tensor_tensor(
                    ov, zv, s0_b[:, r : r + 1], mv,
                    op0=OP.subtract, op1=OP.subtract,
                )
                nc.sync.dma_start(
                    out[b].rearrange("(p j c) -> p j c", p=128, j=NCHUNK)[:, r, :],
                    outt[:, r, :],
                )

        def emit_sample_c(head, z):
            xT, bias_s, t5, rowsum_s = head
            for r in range(NCHUNK):
                lhsT = xT[:, r * 128 : (r + 1) * 128]
                ps = ps_g.tile([128, DIM], F32, tag="gram")
                nc.tensor.matmul(
                    ps[:, 0:512], lhsT, xT[:, 0:512],
                    start=True, stop=False, skip_group_check=True,
                )
                nc.tensor.matmul(
                    ps[:, 512:640], lhsT, xT[:, 512:640],
                    start=True, stop=False, skip_group_check=True,
                )
                for j in range(NCHUNK):
                    nc.tensor.matmul(
                        ps[:, j * 128 : (j + 1) * 128],
                        c_sel[:, j * 128 : (j + 1) * 128], t5[:],
                        start=False, stop=True, skip_group_check=True,
                    )
                nc.scalar.activation(
                    z[:, r, :], ps[:], AF.Sqrt,
                    bias=bias_s[:, r : r + 1],
                    scale=neg2alpha,
                    accum_out=rowsum_s[:, r : r + 1],
                )

        # ---- per-sample software pipeline ----
        # heads 2 samples ahead, squares 4 ahead, tails lag 2 samples so
        # every cross-engine chain has a full iteration of slack.
        xbgs = {0: xbg0, 1: in_dma(1), 2: in_dma(2)}
        sqss = {k: emit_sq(k, xbgs[k // 2], eng=nc.vector) for k in range(4)}
        sqss.update({k: emit_sq(k, xbgs[k // 2]) for k in range(4, 6)})
        heads = {
            0: emit_head(0, xbgs[0], sqss.pop(0)),
            1: emit_head(1, xbgs[0], sqss.pop(1)),
        }
        pend = {}
        for n in range(B_CORE):
            g = n // 2
            if n + 2 < B_CORE:
                g2 = (n + 2) // 2
                heads[n + 2] = emit_head(n + 2, xbgs[g2], sqss.pop(n + 2))
                if n % 2 == 0 and g + 3 < NG:
                    xbgs[g + 3] = in_dma(g + 3)
            if n + 6 < B_CORE:
                sqss[n + 6] = emit_sq(n + 6, xbgs[(n + 6) // 2])
            st = pend.pop(n - 2, None)
            if st is not None:
                tail_prep_a(st)
                tail_prep_b(st)
                tail_stt(st)
            z = zp.tile([128, NCHUNK, DIM], F32, tag="z")
            head = heads.pop(n)
            emit_sample_c(head, z)
            pend[n] = {"n": n, "z": z, "rowsum": head[3]}
            if n == B_CORE - 1:
                st = pend.pop(n - 1)
                tail_prep_a(st)
                tail_prep_b(st)
                tail_stt(st)

        st = pend.pop(B_CORE - 1)
        tail_prep_a(st)
        tail_prep_b(st)
        tail_stt(st)

    nc.compile()
    return nc


def _get_nc():
    global _cached_nc
    if _cached_nc is None:
        _cached_nc = build()
    return _cached_nc


def make_in_maps(x: np.ndarray, t: np.ndarray):
    alpha = float(np.exp(t.astype(np.float64))[0, 0])
    consts = np.zeros((128, 2), dtype=np.float32)
    consts[:, 0] = -2.0 * alpha
    consts[:, 1] = 2.0 * alpha
    ident = np.eye(128, dtype=np.float32)
    sel = np.zeros((2 * NCHUNK, NCHUNK * 128), dtype=np.float32)
    for j in range(NCHUNK):
        sel[j, j * 128 : (j + 1) * 128] = 1.0
        sel[NCHUNK + j, j * 128 : (j + 1) * 128] = 1.0
    xs = x.reshape(N_CORES, B_CORE, DIM, M)
    return [
        {"x": np.ascontiguousarray(xs[c]), "consts": consts, "ident": ident,
         "sel": sel}
        for c in range(N_CORES)
    ]


def kernel(x: np.ndarray, t: np.ndarray) -> np.ndarray:
    x = np.asarray(x, dtype=np.float32)
    t = np.asarray(t, dtype=np.float32)
    nc = _get_nc()
    res = run_bass_kernel_spmd(nc, make_in_maps(x, t), core_ids=list(range(N_CORES)))
    return np.concatenate([r["out"] for r in res.results], axis=0)
